# revision 1
# baseline (speedup 1.0000x reference)
"""Trainium2 Bass kernel for a 4-layer scratch RNN (tanh), data-parallel over batch.

Problem (hardcoded): T=256, B=128, H=1024, L=4, fp32 I/O.
  per layer: xw = X @ W_xh + b ; h_t = tanh(xw_t + h_{t-1} @ W_hh)
  outputs = layer-3 h_t for all t;  hs = stack of each layer's h_{T-1}

Sharding: batch 128 -> 16 per core across 8 NeuronCores, weights replicated.

Device algorithm (per core, everything transposed so H lives on partitions):
  XT buffers A/B in SBUF: [128 part, 8 k-tiles, 4096 token-cols], fp16,
  token col = t*16 + b. Layer l reads XT from one buffer (proj moving operand),
  writes xw into the other, and the scan then updates that same buffer in
  place: step t reads xw col t (identity-matmul into PSUM, starts the
  accumulation group), accumulates 64 W_hh matmuls (W stationary [128,128]
  fp16 tiles, moving operand = state cols t-1, N=16), then one Tanh ACT
  writes state col t back (fp16). Layer 3 additionally writes fp32 tanh
  results to DRAM. fp16 operands + fp32 PSUM accumulation keeps max abs
  error ~1e-3.
"""

import numpy as np

import concourse.bass as bass
import concourse.mybir as mybir
import concourse.tile as tile
from concourse import bacc, bass_utils
from concourse.bass import ts
from concourse.masks import make_identity

F16 = mybir.dt.float16
F32 = mybir.dt.float32
AF = mybir.ActivationFunctionType

T, B, H, L = 256, 128, 1024, 4
N_CORES = 8
BSH = B // N_CORES          # 16 batch per core
NT = T * BSH                # 4096 token columns per core
KT = H // 128               # 8 k/j tiles

_CACHE = {}


def _build():
    nc = bacc.Bacc("TRN2", target_bir_lowering=False, debug=False,
                   num_devices=N_CORES)

    xt0 = nc.dram_tensor("xt0", [128, KT, NT], F16, kind="ExternalInput").ap()
    wxh = nc.dram_tensor("wxh", [L, 128, KT, KT, 128], F16,
                         kind="ExternalInput").ap()
    whh = nc.dram_tensor("whh", [L, 128, KT, KT, 128], F16,
                         kind="ExternalInput").ap()
    bias = nc.dram_tensor("bias", [128, L, KT], F32, kind="ExternalInput").ap()
    yt = nc.dram_tensor("yt", [128, KT, NT], F32, kind="ExternalOutput").ap()
    hlast = nc.dram_tensor("hlast", [L, 128, KT, BSH], F32,
                           kind="ExternalOutput").ap()

    with tile.TileContext(nc) as tc:
        with (
            tc.tile_pool(name="big", bufs=1) as big,
            tc.tile_pool(name="w", bufs=1) as wpool,
            tc.tile_pool(name="small", bufs=1) as small,
            tc.tile_pool(name="st", bufs=4) as st,
            tc.tile_pool(name="psc", bufs=2, space="PSUM") as psc,
            tc.tile_pool(name="psp", bufs=2, space="PSUM") as psp,
        ):
            bufA = big.tile([128, KT, NT], F16, tag="bufA")
            bufB = big.tile([128, KT, NT], F16, tag="bufB")
            bias_sb = small.tile([128, L, KT], F32, tag="bias")
            ident = small.tile([128, 128], F16, tag="ident")

            make_identity(nc, ident[:])
            nc.sync.dma_start(bias_sb[:], bias[:])
            nc.sync.dma_start(bufA[:], xt0[:])

            for l in range(L):
                xin = bufA if l % 2 == 0 else bufB   # proj input
                xst = bufB if l % 2 == 0 else bufA   # xw, then state

                wx = wpool.tile([128, KT, KT, 128], F16, tag="wx")
                wh = wpool.tile([128, KT, KT, 128], F16, tag="wh")
                nc.sync.dma_start(wx[:], wxh[l])
                nc.sync.dma_start(wh[:], whh[l])

                # projection: xst[:, j, :] = sum_i wx[:,i,j,:].T @ xin[:, i, :] + b
                for tt in range(NT // 512):
                    for j in range(KT):
                        ps = psp.tile([128, 512], F32, tag="psp")
                        for i in range(KT):
                            nc.tensor.matmul(
                                ps[:], wx[:, i, j, :], xin[:, i, ts(tt, 512)],
                                start=(i == 0), stop=(i == KT - 1),
                            )
                        nc.scalar.activation(
                            xst[:, j, ts(tt, 512)], ps[:],
                            AF.Identity, bias=bias_sb[:, l, j : j + 1],
                        )

                # scan
                for t in range(T):
                    ps = psc.tile([128, KT * BSH], F32, tag="psc")
                    nc.tensor.matmul(
                        ps[:], ident[:], xst[:, :, ts(t, BSH)],
                        start=True, stop=(t == 0),
                    )
                    if t > 0:
                        for j in range(KT):
                            for i in range(KT):
                                nc.tensor.matmul(
                                    ps[:, ts(j, BSH)], wh[:, i, j, :],
                                    xst[:, i, ts(t - 1, BSH)],
                                    start=False,
                                    stop=(j == KT - 1 and i == KT - 1),
                                )
                    psv = ps.rearrange("p (j b) -> p j b", j=KT)
                    nc.scalar.activation(xst[:, :, ts(t, BSH)], psv, AF.Tanh)
                    if l == L - 1:
                        yst = st.tile([128, KT, BSH], F32, tag="yst")
                        nc.scalar.activation(yst[:], psv, AF.Tanh)
                        nc.sync.dma_start(yt[:, :, ts(t, BSH)], yst[:])
                    if t == T - 1:
                        hst = st.tile([128, KT, BSH], F32, tag="hst")
                        nc.scalar.activation(hst[:], psv, AF.Tanh)
                        nc.sync.dma_start(hlast[l], hst[:])
    nc.compile()
    return nc


def _get_nc():
    if "nc" not in _CACHE:
        _CACHE["nc"] = _build()
    return _CACHE["nc"]


def _prep_core_inputs(inputs, wxh_p, whh_p, bias_p, core):
    xs = inputs[:, core * BSH : (core + 1) * BSH, :]        # [T, BSH, H]
    xt0 = (
        xs.transpose(2, 0, 1).reshape(KT, 128, NT).transpose(1, 0, 2)
    ).astype(np.float16)
    return {"xt0": np.ascontiguousarray(xt0), "wxh": wxh_p, "whh": whh_p,
            "bias": bias_p}


def kernel(inputs, W_xh, W_hh, b_h):
    inputs = np.asarray(inputs, dtype=np.float32)
    W_xh = np.asarray(W_xh, dtype=np.float32)
    W_hh = np.asarray(W_hh, dtype=np.float32)
    b_h = np.asarray(b_h, dtype=np.float32)

    nc = _get_nc()

    def wprep(W):
        # W[l][d_in, h_out] -> [L, 128, d_tile, h_tile, 128] fp16 (lhsT tiles)
        return np.ascontiguousarray(np.stack([
            W[l].reshape(KT, 128, KT, 128).transpose(1, 0, 2, 3)
            for l in range(L)
        ]).astype(np.float16))

    wxh_p = wprep(W_xh)
    whh_p = wprep(W_hh)
    bias_p = np.ascontiguousarray(
        b_h.reshape(L, KT, 128).transpose(2, 0, 1).astype(np.float32))

    in_maps = [_prep_core_inputs(inputs, wxh_p, whh_p, bias_p, c)
               for c in range(N_CORES)]
    res = bass_utils.run_bass_kernel_spmd(nc, in_maps,
                                          core_ids=list(range(N_CORES)))

    outs = np.empty((T, B, H), dtype=np.float32)
    hs = np.empty((L, B, H), dtype=np.float32)
    for c in range(N_CORES):
        ytc = res.results[c]["yt"]           # [128, KT, NT]
        hlc = res.results[c]["hlast"]        # [L, 128, KT, BSH]
        bsl = slice(c * BSH, (c + 1) * BSH)
        outs[:, bsl, :] = (
            ytc.transpose(1, 0, 2).reshape(H, T, BSH).transpose(1, 2, 0))
        hs[:, bsl, :] = (
            hlc.transpose(0, 2, 1, 3).reshape(L, H, BSH).transpose(0, 2, 1))
    return outs, hs


# revision 5
# speedup vs baseline: 6477.4057x; 6477.4057x over previous
"""Trainium2 Bass kernel for a 4-layer scratch RNN (tanh), data-parallel over batch.

Problem (hardcoded): T=256, B=128, H=1024, L=4, fp32 I/O.
  per layer: xw = X @ W_xh + b ; h_t = tanh(xw_t + h_{t-1} @ W_hh)
  outputs = layer-3 h_t for all t;  hs = stack of each layer's h_{T-1}

Sharding: batch 128 -> 16 per core across 8 NeuronCores, weights replicated.

Device algorithm (per core, everything transposed so H lives on partitions):
  XT buffers A/B in SBUF: [128 part, 8 k-tiles, 4096 token-cols], fp16,
  token col = t*16 + b. Layer l reads XT from one buffer (proj moving operand),
  writes xw into the other, and the scan then updates that same buffer in
  place: step t reads xw col t (identity-matmul into PSUM, starts the
  accumulation group), accumulates 64 W_hh matmuls (W stationary [128,128]
  fp16 tiles, moving operand = state cols t-1, N=16), then one Tanh ACT
  writes state col t back (fp16). Layer 3 additionally writes fp32 tanh
  results to DRAM. fp16 operands + fp32 PSUM accumulation keeps max abs
  error ~1e-3.
"""

import numpy as np

import concourse.bass as bass
import concourse.mybir as mybir
import concourse.tile as tile
from concourse import bacc, bass_utils
from concourse.bass import ts
from concourse.masks import make_identity

F16 = mybir.dt.float16
F32 = mybir.dt.float32
AF = mybir.ActivationFunctionType

T, B, H, L = 256, 128, 1024, 4
N_CORES = 8
BSH = B // N_CORES          # 16 batch per core
NT = T * BSH                # 4096 token columns per core
KT = H // 128               # 8 k/j tiles

_CACHE = {}


def _build(split_scan=False):
    nc = bacc.Bacc("TRN2", target_bir_lowering=False, debug=False,
                   num_devices=N_CORES)

    xt0 = nc.dram_tensor("xt0", [128, KT, NT], F16, kind="ExternalInput").ap()
    wxh = nc.dram_tensor("wxh", [L, 128, KT, KT, 128], F16,
                         kind="ExternalInput").ap()
    whh = nc.dram_tensor("whh", [L, 128, KT, KT, 128], F16,
                         kind="ExternalInput").ap()
    bias = nc.dram_tensor("bias", [128, L, KT], F32, kind="ExternalInput").ap()
    yt = nc.dram_tensor("yt", [128, KT, NT], F32, kind="ExternalOutput").ap()
    hlast = nc.dram_tensor("hlast", [L, 128, KT, BSH], F32,
                           kind="ExternalOutput").ap()

    with tile.TileContext(nc) as tc:
        with (
            tc.tile_pool(name="big", bufs=1) as big,
            tc.tile_pool(name="w", bufs=1) as wpool,
            tc.tile_pool(name="small", bufs=1) as small,
            tc.tile_pool(name="st", bufs=4) as st,
            tc.tile_pool(name="psc", bufs=2, space="PSUM") as psc,
            tc.tile_pool(name="psp", bufs=2, space="PSUM") as psp,
        ):
            bufA = big.tile([128, KT, NT], F16, tag="bufA")
            bufB = big.tile([128, KT, NT], F16, tag="bufB")
            bias_sb = small.tile([128, L, KT], F32, tag="bias")
            ident = small.tile([128, 128], F16, tag="ident")

            make_identity(nc, ident[:])
            nc.sync.dma_start(bias_sb[:], bias[:])
            nc.sync.dma_start(bufA[:], xt0[:])

            for l in range(L):
                xin = bufA if l % 2 == 0 else bufB   # proj input
                xst = bufB if l % 2 == 0 else bufA   # xw, then state

                wx = wpool.tile([128, KT, KT, 128], F16, tag="wx")
                wh = wpool.tile([128, KT, KT, 128], F16, tag="wh")
                nc.sync.dma_start(wx[:], wxh[l])
                nc.sync.dma_start(wh[:], whh[l])

                # projection: xst[:, j, :] = sum_i wx[:,i,j,:].T @ xin[:, i, :] + b
                for tt in range(NT // 512):
                    for j in range(KT):
                        ps = psp.tile([128, 512], F32, tag="psp")
                        for i in range(KT):
                            nc.tensor.matmul(
                                ps[:], wx[:, i, j, :], xin[:, i, ts(tt, 512)],
                                start=(i == 0), stop=(i == KT - 1),
                            )
                        nc.scalar.activation(
                            xst[:, j, ts(tt, 512)], ps[:],
                            AF.Identity, bias=bias_sb[:, l, j : j + 1],
                        )

                # scan
                if split_scan:
                    # Two PSUM banks per step (j halves). tanh of the first
                    # half fires mid-step, so step t+1's matmuls never wait on
                    # ACT latency (PSUM bank P10 serialization is per bank).
                    HK = KT // 2
                    for t in range(T):
                        yst = None
                        hst = None
                        if l == L - 1:
                            yst = st.tile([128, KT, BSH], F32, tag="yst")
                        if t == T - 1:
                            hst = st.tile([128, KT, BSH], F32, tag="hst")
                        for h2 in range(2):
                            ps = psc.tile([128, HK * BSH], F32, tag="psc")
                            jlo = h2 * HK
                            nc.tensor.matmul(
                                ps[:], ident[:],
                                xst[:, jlo : jlo + HK, ts(t, BSH)],
                                start=True, stop=(t == 0),
                            )
                            if t > 0:
                                for j in range(jlo, jlo + HK):
                                    for i in range(KT):
                                        nc.tensor.matmul(
                                            ps[:, ts(j - jlo, BSH)],
                                            wh[:, i, j, :],
                                            xst[:, i, ts(t - 1, BSH)],
                                            start=False,
                                            stop=(j == jlo + HK - 1
                                                  and i == KT - 1),
                                        )
                            psv = ps.rearrange("p (j b) -> p j b", j=HK)
                            nc.scalar.activation(
                                xst[:, jlo : jlo + HK, ts(t, BSH)], psv,
                                AF.Tanh)
                            if yst is not None:
                                nc.scalar.activation(
                                    yst[:, jlo : jlo + HK, :], psv, AF.Tanh)
                            if hst is not None:
                                nc.scalar.activation(
                                    hst[:, jlo : jlo + HK, :], psv, AF.Tanh)
                        if yst is not None:
                            nc.sync.dma_start(yt[:, :, ts(t, BSH)], yst[:])
                        if hst is not None:
                            nc.sync.dma_start(hlast[l], hst[:])
                else:
                    for t in range(T):
                        ps = psc.tile([128, KT * BSH], F32, tag="psc")
                        nc.tensor.matmul(
                            ps[:], ident[:], xst[:, :, ts(t, BSH)],
                            start=True, stop=(t == 0),
                        )
                        if t > 0:
                            for j in range(KT):
                                for i in range(KT):
                                    nc.tensor.matmul(
                                        ps[:, ts(j, BSH)], wh[:, i, j, :],
                                        xst[:, i, ts(t - 1, BSH)],
                                        start=False,
                                        stop=(j == KT - 1 and i == KT - 1),
                                    )
                        psv = ps.rearrange("p (j b) -> p j b", j=KT)
                        nc.scalar.activation(xst[:, :, ts(t, BSH)], psv, AF.Tanh)
                        if l == L - 1:
                            yst = st.tile([128, KT, BSH], F32, tag="yst")
                            nc.scalar.activation(yst[:], psv, AF.Tanh)
                            nc.sync.dma_start(yt[:, :, ts(t, BSH)], yst[:])
                        if t == T - 1:
                            hst = st.tile([128, KT, BSH], F32, tag="hst")
                            nc.scalar.activation(hst[:], psv, AF.Tanh)
                            nc.sync.dma_start(hlast[l], hst[:])
    nc.compile()
    return nc


def _get_nc():
    if "nc" not in _CACHE:
        _CACHE["nc"] = _build()
    return _CACHE["nc"]


def _prep_core_inputs(inputs, wxh_p, whh_p, bias_p, core):
    xs = inputs[:, core * BSH : (core + 1) * BSH, :]        # [T, BSH, H]
    xt0 = (
        xs.transpose(2, 0, 1).reshape(KT, 128, NT).transpose(1, 0, 2)
    ).astype(np.float16)
    return {"xt0": np.ascontiguousarray(xt0), "wxh": wxh_p, "whh": whh_p,
            "bias": bias_p}


def kernel(inputs, W_xh, W_hh, b_h):
    inputs = np.asarray(inputs, dtype=np.float32)
    W_xh = np.asarray(W_xh, dtype=np.float32)
    W_hh = np.asarray(W_hh, dtype=np.float32)
    b_h = np.asarray(b_h, dtype=np.float32)

    nc = _get_nc()

    def wprep(W):
        # W[l][d_in, h_out] -> [L, 128, d_tile, h_tile, 128] fp16 (lhsT tiles)
        return np.ascontiguousarray(np.stack([
            W[l].reshape(KT, 128, KT, 128).transpose(1, 0, 2, 3)
            for l in range(L)
        ]).astype(np.float16))

    wxh_p = wprep(W_xh)
    whh_p = wprep(W_hh)
    bias_p = np.ascontiguousarray(
        b_h.reshape(L, KT, 128).transpose(2, 0, 1).astype(np.float32))

    in_maps = [_prep_core_inputs(inputs, wxh_p, whh_p, bias_p, c)
               for c in range(N_CORES)]
    res = bass_utils.run_bass_kernel_spmd(nc, in_maps,
                                          core_ids=list(range(N_CORES)))

    outs = np.empty((T, B, H), dtype=np.float32)
    hs = np.empty((L, B, H), dtype=np.float32)
    for c in range(N_CORES):
        ytc = res.results[c]["yt"]           # [128, KT, NT]
        hlc = res.results[c]["hlast"]        # [L, 128, KT, BSH]
        bsl = slice(c * BSH, (c + 1) * BSH)
        outs[:, bsl, :] = (
            ytc.transpose(1, 0, 2).reshape(H, T, BSH).transpose(1, 2, 0))
        hs[:, bsl, :] = (
            hlc.transpose(0, 2, 1, 3).reshape(L, H, BSH).transpose(0, 2, 1))
    return outs, hs
